# revision 26
# baseline (speedup 1.0000x reference)
"""Sparse-attention Trainium2 kernel (nn_Attention_44341242364527).

Strategy
--------
Head-tensor-parallel over 8 NeuronCores (2 heads/core, Megatron-style:
Wq/Wk/Wv column-sharded, Wo row-sharded, partial outputs all-reduced on
the host during unshard).

The sparse gather ``k[idx]`` / ``v[idx]`` is reformulated densely: since
``exp(qk/sqrt(D) + geo) = exp(qk/sqrt(D)) * exp(geo)``, and idx/valid/
geo_bias are host-known inputs, the host pre-scatters

    WT[h][s', s] = sum_k 1[idx[s,k]==s' & valid & s'<=s] * exp(geo[h,s,k])

Then per head, on device (everything transposed so no on-chip transposes
are needed):

    ST  = Kh @ Qh.T                  [s', s]   (dense scores)
    AT  = exp(ST/sqrt(D)) * WT       [s', s]   (un-normalized attention)
    AOT = Vh.T @ AT                  [d, s]    (un-normalized context)
    Z   = colsum(AT)                 [1, s]    (softmax denominator)
    Y  += (AOT/Z).T @ WoT_shard      [s, HID]  (partial output)

Causality makes AT block-lower-triangular: only ~62% of blocks are
computed. WT==0 kills both the masked and the un-selected entries.

Changes vs the 193us baseline (measured via burst-slope timing, the only
reliable clock through the noisy axon tunnel):
  - Z (the column sum of AT over s') fully off the PE: AT tiles are
    tree-accumulated on the DVE (bf16, 2x mode); the cross-partition
    column sum runs on the idle GpSimd engine (partition_all_reduce,
    which also leaves the result broadcast on all 128 partitions), then
    a DVE reciprocal feeds the normalize multiply directly. No Z
    matmuls on the PE at all (~8k cycles/rep) and one PSUM bank freed,
    which doubles the output-projection PSUM ring (psY bufs=2) and
    removes the bank-recycle stalls there.
  - All matmul operands bf16 (was float32r): no fp32r narrow-stream 4x
    penalty, FWL on stationary loads, 2x DVE modes on the exp*WT multiply.
  - Both heads packed into ONE [128,1024] AT tile per s'-tile: the exp*WT
    multiply and Z tree-add run as single wide DVE ops, halving attention
    instruction count and cross-engine handoffs (hardware charges
    ~150-200ns of sync/dispatch per instruction that the cost model does
    not show).
  - Stage-separated PSUM pools (proj/st/attn-out/z/y): a shared pool ring
    serializes pipeline stages in program order and blocks cross-chunk
    overlap.
  - Batched DMA: hsT in [128, 4x512] k-groups, WT head-interleaved in
    [128, 4x1024] t-groups, y one row-block [128, 2048] per tile; the
    attention aop is staged to SBUF immediately so its PSUM bank recycles
    without waiting for the Z chain.
  - hs/WT pre-tiled on the host into exact SBUF-tile order (hs4/wt4):
    every steady-state input DMA is one fully-contiguous DRAM block with
    4-8 KB per-partition lines (was 1 KB pieces with strided descriptors).
"""

import math
import sys

sys.path.insert(0, "/opt/trn_rl_repo")

import numpy as np

B, S, H, D, KS = 1, 2048, 16, 128, 64
HID = H * D
NCORES = 8
HPC = H // NCORES          # heads per core
CPC = HPC * D              # output channels per core
P = 128                    # partitions
SC = 512                   # s-chunk (PSUM bank width in f32)
NJ = S // SC               # 4 s-chunks
NT = S // P                # 16 s'-tiles
NK = HID // P              # 16 contraction chunks

# dtype knobs (numpy dtype name per tensor class); PSUM is always f32.
DT_PROJ = "bfloat16"       # hsT + Wq/Wk/Wv operands of the QKV projections
DT_QK = "bfloat16"         # Q^T/K^T operands of the score matmul
DT_ATT = "bfloat16"        # exp(S)*W and V operands of the AV matmul
DT_WT = "bfloat16"         # the scattered exp(geo) tensor (DMA-heavy)
DT_WO = "bfloat16"         # AOT and Wo operands of the output projection

_CACHE = {}


def _np_dt(name):
    if name == "bfloat16":
        import ml_dtypes

        return np.dtype(ml_dtypes.bfloat16)
    if name == "float32r":
        return np.dtype(np.float32)
    return np.dtype(name)


def _my_dt(name):
    from concourse import mybir

    return {
        "float32": mybir.dt.float32,
        "float32r": mybir.dt.float32r,
        "bfloat16": mybir.dt.bfloat16,
    }[name]


def _build_nc(reps=1, loop=None, mode="full"):
    """Build the kernel NEFF. reps: python-unrolled body repetitions
    (pipelined back-to-back, used for marginal timing). loop: optional
    hardware For_i loop count around the whole body (huge amplification
    for noise-immune wall-clock timing; each iteration re-runs the
    reps-unrolled body, with an all-engine barrier between iterations)."""
    import contextlib

    import concourse.tile as tile
    from concourse import bacc, bass_isa, mybir

    F32 = mybir.dt.float32
    EXP = mybir.ActivationFunctionType.Exp
    MULT = mybir.AluOpType.mult
    ADD = mybir.AluOpType.add
    RADD = bass_isa.ReduceOp.add

    nc = bacc.Bacc("TRN2", target_bir_lowering=False, debug=False,
                   num_devices=NCORES)

    # hs4/wt4 are host-pre-tiled into exact SBUF-tile order so every
    # steady-state DMA is a fully contiguous block (4-8 KB per partition
    # line instead of 1 KB pieces): hs4[j, g] is the [P, 4*SC] hsT
    # k-group tile for chunk j; wt4[j, g] is the [P, 4t x (h, SC)]
    # head-interleaved WT t-group tile for chunk j.
    hs4 = nc.dram_tensor("hs4", [NJ, NK // 4, P, 4 * SC], _my_dt(DT_PROJ),
                         kind="ExternalInput")
    wqT = nc.dram_tensor("wqT", [HID, CPC], _my_dt(DT_PROJ), kind="ExternalInput")
    wkT = nc.dram_tensor("wkT", [HID, CPC], _my_dt(DT_PROJ), kind="ExternalInput")
    wvT = nc.dram_tensor("wvT", [HID, CPC], _my_dt(DT_PROJ), kind="ExternalInput")
    woT = nc.dram_tensor("woT", [CPC, HID], _my_dt(DT_WO), kind="ExternalInput")
    wt4 = nc.dram_tensor("wt4", [NJ, NT // 4, P, 4 * 2 * SC], _my_dt(DT_WT),
                         kind="ExternalInput")
    y = nc.dram_tensor("y", [S, HID], mybir.dt.bfloat16, kind="ExternalOutput")

    inv_sqrt_d = 1.0 / math.sqrt(D)

    def mm(out, lhsT, rhs, **kw):
        nc.tensor.matmul(out, lhsT, rhs, **kw)

    with tile.TileContext(nc) as tc, \
            nc.allow_low_precision(reason="bf16 matmul operands; PSUM accum stays f32"):
        with tc.tile_pool(name="persist", bufs=1) as persist:
            QT = [persist.tile([P, S], _my_dt(DT_QK), tag=f"qt{h}", name=f"qt{h}")
                  for h in range(HPC)]
            KT = [persist.tile([P, S], _my_dt(DT_QK), tag=f"kt{h}", name=f"kt{h}")
                  for h in range(HPC)]
            Vsb = [persist.tile([P, CPC], _my_dt(DT_ATT), tag=f"v{t}", name=f"vres{t}")
                   for t in range(NT)]
            AOT = [persist.tile([P, S], _my_dt(DT_WO), tag=f"aot{h}", name=f"aot{h}")
                   for h in range(HPC)]
            if mode == "noattn":
                for h in range(HPC):
                    nc.gpsimd.memset(AOT[h][:], 0.5)
            if mode == "noproj":
                for h in range(HPC):
                    nc.gpsimd.memset(QT[h][:], 0.01)
                    nc.gpsimd.memset(KT[h][:], 0.01)
                for t in range(NT):
                    nc.gpsimd.memset(Vsb[t][:], 0.01)
            # diagnostic modes: "dmaonly" emits only the steady-state DMA
            # streams (hs4/wt4 in, y out); "noindma" replaces hs4/wt4 DMAs
            # with fixed memset tiles but keeps all compute + y-write.
            hs_fix, wt_fix, ys_fix = [], [], []
            if mode == "noindma":
                for g in range(NK // 4):
                    t_ = persist.tile([P, 4 * SC], _my_dt(DT_PROJ),
                                      tag=f"hsfix{g}", name=f"hsfix{g}")
                    nc.gpsimd.memset(t_[:], 0.01)
                    hs_fix.append(t_)
                for g in range(NT // 4):
                    t_ = persist.tile([P, 4 * 2 * SC], _my_dt(DT_WT),
                                      tag=f"wtfix{g}", name=f"wtfix{g}")
                    nc.gpsimd.memset(t_[:], 0.01)
                    wt_fix.append(t_)
            if mode == "dmaonly":
                t_ = persist.tile([P, S], mybir.dt.bfloat16, tag="ysfix",
                                  name="ysfix")
                nc.gpsimd.memset(t_[:], 0.01)
                ys_fix.append(t_)

            # Pipelined over j-chunks of the query/sequence dim: for each j,
            # project chunk j, run attention for chunk j (both heads), then
            # the output projection for s-tiles 4j..4j+3. Chunk-level deps
            # let the Tile scheduler overlap all three stages across j.
            with tc.tile_pool(name="wpool", bufs=1) as wpool, \
                 tc.tile_pool(name="hpool", bufs=6) as hpool, \
                 tc.tile_pool(name="wop", bufs=1) as wop, \
                 tc.tile_pool(name="wtp", bufs=5) as wtp, \
                 tc.tile_pool(name="atp", bufs=10) as atp, \
                 tc.tile_pool(name="rbp", bufs=4) as rbp, \
                 tc.tile_pool(name="zacc", bufs=3) as zaccp, \
                 tc.tile_pool(name="ypool", bufs=4) as ypool, \
                 tc.tile_pool(name="psP", bufs=2, space="PSUM") as psP, \
                 tc.tile_pool(name="psS", bufs=2, space="PSUM") as psS, \
                 tc.tile_pool(name="psA", bufs=2, space="PSUM") as psA, \
                 tc.tile_pool(name="psY", bufs=2, space="PSUM") as psY:
                wq_sb, wk_sb, wv_sb = [], [], []
                wo_sb = []

                def emit_wo():
                    for h in range(HPC):
                        t_ = wop.tile([P, HID], _my_dt(DT_WO),
                                      tag=f"wo{h}", name=f"wo{h}")
                        nc.sync.dma_start(t_[:], woT[h * P:(h + 1) * P, :])
                        wo_sb.append(t_)

                def gen_proj(j, first):
                    # -- QKV projection for chunk j (generator: yields at
                    # ~1.7us PE-work quanta so the driver can interleave) --
                    # DMA order matters for the cold start: the first
                    # Q-projection chain needs all wq chunks + all hst
                    # chunks of j=0, so those stream first; wk/wv/wo
                    # follow (they are consumed by later chains).
                    hs_t = []
                    for k in range(NK):
                        if first and mode != "dmaonly":
                            t_ = wpool.tile([P, CPC], _my_dt(DT_PROJ),
                                            tag=f"wq{k}", name=f"wq{k}")
                            nc.sync.dma_start(t_[:], wqT[k * P:(k + 1) * P, :])
                            wq_sb.append(t_)
                        if k % 4 == 0:
                            if mode == "noindma":
                                tb = hs_fix[k // 4]
                            else:
                                tb = hpool.tile([P, 4 * SC], _my_dt(DT_PROJ),
                                                tag="hst", name="hst")
                                nc.sync.dma_start(tb[:], hs4[j, k // 4])
                        hs_t.append(tb[:, (k % 4) * SC:(k % 4 + 1) * SC])
                    if mode == "dmaonly":
                        return
                    if first:
                        for lst, dram, nm in ((wk_sb, wkT, "wk"),
                                              (wv_sb, wvT, "wv")):
                            for k in range(NK):
                                t_ = wpool.tile([P, CPC], _my_dt(DT_PROJ),
                                                tag=f"{nm}{k}", name=f"{nm}{k}")
                                nc.sync.dma_start(
                                    t_[:], dram[k * P:(k + 1) * P, :])
                                lst.append(t_)
                        emit_wo()
                    yield
                    # q-chains before k-chains: the k-copies WAR against the
                    # PREVIOUS rep's attn(3) stationary reads; emitting them
                    # later keeps that wait off the ACT/DVE queue head.
                    for wi, (w_sb, acc) in enumerate(((wq_sb, QT),
                                                      (wk_sb, KT))):
                        for h in range(HPC):
                            pp = psP.tile([P, SC], F32, tag="big", name="ps_proj")
                            for k in range(NK):
                                mm(pp[:], w_sb[k][:, h * D:(h + 1) * D],
                                   hs_t[k][:],
                                   start=(k == 0), stop=(k == NK - 1))
                                if k == NK // 2 - 1:
                                    yield
                            if (2 * h + wi) % 2 == 0:
                                nc.scalar.copy(
                                    acc[h][:, j * SC:(j + 1) * SC], pp[:])
                            else:
                                nc.vector.tensor_copy(
                                    acc[h][:, j * SC:(j + 1) * SC], pp[:])
                            yield
                    for si in range(SC // P):
                        vp = psP.tile([P, CPC], F32, tag="big", name="ps_projv")
                        for k in range(NK):
                            mm(vp[:], hs_t[k][:, si * P:(si + 1) * P],
                               wv_sb[k][:],
                               start=(k == 0), stop=(k == NK - 1))
                            if k == NK // 2 - 1:
                                yield
                        nc.vector.tensor_copy(Vsb[4 * j + si][:], vp[:])
                        yield

                def gen_attn(j):
                    # -- attention for chunk j. Both heads are packed into
                    # ONE wide [128, 1024] AT tile per s'-tile t (h0 cols
                    # 0:512, h1 cols 512:1024, chunk-aligned): the exp*WT
                    # multiply and the Z tree-add run as single wide DVE ops,
                    # halving instruction count and cross-engine handoffs,
                    # which is what the hardware actually charges for. WT is
                    # DMA'd head-interleaved to match. Z: DVE tree-accumulates
                    # AT into zacc; a [1,512] ones-matmul per head forms the
                    # column sum; reciprocal; a broadcast ones-matmul spreads
                    # 1/Z for the normalize multiply. --
                    tmax = min(4 * j + 3, NT - 1)
                    if mode != "dmaonly":
                        aop = [psA.tile([P, SC], F32, tag="ao", name=f"ao{h}")
                               for h in range(HPC)]
                        zacc = zaccp.tile([P, 2 * SC], _my_dt(DT_ATT),
                                          tag="za", name="za")
                    wtg = {}
                    for g in range(0, tmax + 1, 4):
                        if mode == "noindma":
                            wtg[g] = wt_fix[g // 4]
                            continue
                        t_ = wtp.tile([P, 4 * 2 * SC], _my_dt(DT_WT),
                                      tag="wt", name="wt")
                        nc.sync.dma_start(t_[:], wt4[j, g // 4])
                        wtg[g] = t_
                    if mode == "dmaonly":
                        return
                    yield
                    pend = []

                    def drain_one():
                        t_, at_, o_ = pend.pop(0)
                        for h in range(HPC):
                            mm(aop[h][:, o_:SC],
                               Vsb[t_][:, h * D:(h + 1) * D],
                               at_[:, h * SC + o_:(h + 1) * SC],
                               start=(t_ == 0), stop=(t_ == tmax))

                    for t in range(tmax + 1):
                        # within the diagonal block only columns s >= 128t
                        # are causally reachable; compute [o:SC) per head and
                        # zero-fill the acausal strip so the wide tree ops
                        # stay exact (WT is zero there anyway).
                        o = max(0, t * P - j * SC)
                        at = atp.tile([P, 2 * SC], _my_dt(DT_ATT), tag="at",
                                      name="at")
                        for h in range(HPC):
                            stp = psS.tile([P, SC], F32, tag="st", name="st")
                            mm(stp[:, o:SC], KT[h][:, t * P:(t + 1) * P],
                               QT[h][:, j * SC + o:(j + 1) * SC],
                               start=True, stop=True)
                            if o:
                                nc.gpsimd.memset(at[:, h * SC:h * SC + o], 0.0)
                            nc.scalar.activation(at[:, h * SC + o:(h + 1) * SC],
                                                 stp[:, o:SC], EXP,
                                                 scale=inv_sqrt_d)
                        g = (t // 4) * 4
                        base = (t - g) * 2 * SC
                        nc.vector.tensor_mul(at[:], at[:],
                                             wtg[g][:, base:base + 2 * SC])
                        if t == 0:
                            nc.vector.tensor_copy(zacc[:], at[:])
                        else:
                            nc.vector.tensor_tensor(zacc[:], zacc[:], at[:],
                                                    ADD)
                        pend.append((t, at, o))
                        yield
                        if len(pend) >= 3:
                            drain_one()
                            yield
                    while pend:
                        drain_one()
                        yield

                    # Z = colsum(AT) over partitions on the (idle) GpSimd
                    # engine; the all-reduce leaves the sum broadcast on all
                    # 128 partitions, so 1/Z feeds the normalize multiply
                    # directly — no PE ones-matmuls, no psZ PSUM bank.
                    zsum = zaccp.tile([P, 2 * SC], _my_dt(DT_ATT), tag="zs",
                                      name="zs")
                    nc.gpsimd.partition_all_reduce(zsum[:], zacc[:], P, RADD)
                    rinv = rbp.tile([P, 2 * SC], _my_dt(DT_ATT), tag="ri",
                                    name="ri")
                    nc.vector.reciprocal(rinv[:], zsum[:])
                    for h in range(HPC):
                        aosb = rbp.tile([P, SC], F32, tag="aosb", name="aosb")
                        nc.scalar.copy(aosb[:], aop[h][:])
                        nc.vector.tensor_tensor(
                            AOT[h][:, j * SC:(j + 1) * SC], aosb[:],
                            rinv[:, h * SC:(h + 1) * SC], MULT)

                def gen_outproj(j):
                    # -- output projection for s-tiles of chunk j --
                    for m in range(4 * j, 4 * j + 4):
                        if mode == "dmaonly":
                            nc.sync.dma_start(y[m * P:(m + 1) * P, :],
                                              ys_fix[0][:])
                            continue
                        ysb = ypool.tile([P, S], mybir.dt.bfloat16,
                                         tag="ysb", name="ysb")
                        for n in range(NJ):
                            yps = psY.tile([P, SC], F32, tag="y", name="ps_y")
                            for h in range(HPC):
                                mm(yps[:], AOT[h][:, m * P:(m + 1) * P],
                                   wo_sb[h][:, n * SC:(n + 1) * SC],
                                   start=(h == 0), stop=(h == HPC - 1))
                            if n % 2 == 0:
                                nc.scalar.copy(
                                    ysb[:, n * SC:(n + 1) * SC], yps[:])
                            else:
                                nc.vector.tensor_copy(
                                    ysb[:, n * SC:(n + 1) * SC], yps[:])
                            if n == 1:
                                yield
                        nc.sync.dma_start(y[m * P:(m + 1) * P, :], ysb[:])
                        yield

                # Emission order IS per-engine execution order (in-order
                # queues), so the driver below round-robins the stage
                # generators: every attention quantum (whose operands ride
                # a PE->ACT->DVE chain) is followed by independent
                # projection/output-projection matmuls, so the PE queue
                # head always has ready work and HAM never re-throttles.
                # Slot j of a rep interleaves attn(j) [proj(j) completed
                # last slot], proj(j+1) [proj(0) of the NEXT rep in the
                # last slot], and outproj(j-1) [carried one slot so the
                # Z/normalize chain of chunk j-1 is long done].
                def drain(gens):
                    gens = [g for g in gens if g is not None]
                    while gens:
                        alive = []
                        for g in gens:
                            try:
                                next(g)
                                alive.append(g)
                            except StopIteration:
                                pass
                        gens = alive

                carry = {"out": None, "proj": None}

                def emit_rep(first, last):
                    if mode == "noproj" and first:
                        emit_wo()
                    for j in range(NJ):
                        gens = []
                        if mode != "noattn":
                            if first and j == 0 and mode != "noproj":
                                # cold start: attn(0) needs proj(0) complete
                                drain([gen_proj(0, True)])
                            gens.append(gen_attn(j))
                        elif first and j == 0 and mode != "noproj":
                            gens.append(gen_proj(0, True))
                        if mode != "noproj":
                            if j + 1 < NJ:
                                gens.append(gen_proj(j + 1, False))
                            elif not last:
                                gens.append(gen_proj(0, False))
                        gens.append(carry["out"])
                        carry["out"] = None
                        drain(gens)
                        carry["out"] = gen_outproj(j)
                    if last or loop is not None:
                        drain([carry["out"]])
                        carry["out"] = None

                loop_ctx = (tc.For_i(0, loop) if loop is not None
                            else contextlib.nullcontext())
                with loop_ctx:
                    for _rep in range(reps):
                        emit_rep(_rep == 0, _rep == reps - 1)

    nc.compile()
    return nc


def _get_nc():
    if "nc" not in _CACHE:
        _CACHE["nc"] = _build_nc()
    return _CACHE["nc"]


def make_in_maps(hidden_states, idx, valid, geo_bias, Wq, Wk, Wv, Wo):
    """Host-side sharding/layout prep: returns the 8 per-core input maps."""
    hs = np.ascontiguousarray(np.asarray(hidden_states, np.float32)[0])
    idx = np.asarray(idx).astype(np.int64)
    valid = np.asarray(valid).astype(bool)

    dt_proj, dt_wo, dt_wt = _np_dt(DT_PROJ), _np_dt(DT_WO), _np_dt(DT_WT)

    hsT = np.ascontiguousarray(hs.T).astype(dt_proj)       # [HID, S]
    # pre-tile into SBUF order: hs4[j, g, p, kk, c] = hsT[128*(4g+kk)+p,
    # 512j+c]  ->  every steady-state DMA is one contiguous block.
    hs4 = np.ascontiguousarray(
        hsT.reshape(NK // 4, 4, P, NJ, SC).transpose(3, 0, 2, 1, 4)
        .reshape(NJ, NK // 4, P, 4 * SC))

    srange = np.arange(S)
    cmask = ((idx <= srange[:, None]) & valid).ravel()
    flat = (idx * S + srange[:, None]).ravel()[cmask]
    eg = np.exp(np.asarray(geo_bias, np.float64))          # [H, S, K]

    in_maps = []
    for c in range(NCORES):
        h0 = HPC * c
        sl = slice(h0 * D, (h0 + HPC) * D)
        wt_c = np.empty((HPC, S, S), dt_wt)
        for hh in range(HPC):
            wt_c[hh] = (np.bincount(flat,
                                    weights=eg[h0 + hh].ravel()[cmask],
                                    minlength=S * S)
                        .reshape(S, S).astype(dt_wt))
        # wt4[j, g, p, tt, h, c] = wt_c[h, 128*(4g+tt)+p, 512j+c]
        wt4 = np.ascontiguousarray(
            wt_c.reshape(HPC, NT // 4, 4, P, NJ, SC)
            .transpose(4, 1, 3, 2, 0, 5)
            .reshape(NJ, NT // 4, P, 4 * 2 * SC))
        in_maps.append({
            "hs4": hs4,
            "wqT": np.ascontiguousarray(np.asarray(Wq)[sl].T).astype(dt_proj),
            "wkT": np.ascontiguousarray(np.asarray(Wk)[sl].T).astype(dt_proj),
            "wvT": np.ascontiguousarray(np.asarray(Wv)[sl].T).astype(dt_proj),
            "woT": np.ascontiguousarray(np.asarray(Wo)[:, sl].T).astype(dt_wo),
            "wt4": wt4,
        })
    return in_maps


def kernel(hidden_states, idx, valid, geo_bias, Wq, Wk, Wv, Wo, bo):
    from concourse import bass_utils

    nc = _get_nc()
    in_maps = make_in_maps(hidden_states, idx, valid, geo_bias, Wq, Wk, Wv, Wo)
    res = bass_utils.run_bass_kernel_spmd(nc, in_maps,
                                          core_ids=list(range(NCORES)))
    out = np.zeros((S, HID), np.float32)
    for r in res.results:
        out += r["y"].astype(np.float32)
    out += np.asarray(bo, np.float32)
    return out.reshape(B, S, HID)



# revision 35
# speedup vs baseline: 1.1087x; 1.1087x over previous
"""Sparse-attention Trainium2 kernel (nn_Attention_44341242364527).

Strategy
--------
Head-tensor-parallel over 8 NeuronCores (2 heads/core, Megatron-style:
Wq/Wk/Wv column-sharded, Wo row-sharded, partial outputs all-reduced on
the host during unshard).

The sparse gather ``k[idx]`` / ``v[idx]`` is reformulated densely: since
``exp(qk/sqrt(D) + geo) = exp(qk/sqrt(D)) * exp(geo)``, and idx/valid/
geo_bias are host-known inputs, the host pre-scatters

    WT[h][s', s] = sum_k 1[idx[s,k]==s' & valid & s'<=s] * exp(geo[h,s,k])

Then per head, on device (everything transposed so no on-chip transposes
are needed):

    ST  = Kh @ Qh.T                  [s', s]   (dense scores)
    AT  = exp(ST/sqrt(D)) * WT       [s', s]   (un-normalized attention)
    AOT = Vh.T @ AT                  [d, s]    (un-normalized context)
    Z   = colsum(AT)                 [1, s]    (softmax denominator)
    Y  += (AOT/Z).T @ WoT_shard      [s, HID]  (partial output)

Causality makes AT block-lower-triangular: only ~62% of blocks are
computed. WT==0 kills both the masked and the un-selected entries.

Changes vs the 193us baseline (measured via burst-slope timing, the only
reliable clock through the noisy axon tunnel):
  - Z (the column sum of AT over s') fully off the PE: AT tiles are
    tree-accumulated on the DVE (bf16, 2x mode); the cross-partition
    column sum runs on the idle GpSimd engine (partition_all_reduce,
    which also leaves the result broadcast on all 128 partitions), then
    a DVE reciprocal feeds the normalize multiply directly. No Z
    matmuls on the PE at all (~8k cycles/rep) and one PSUM bank freed,
    which doubles the output-projection PSUM ring (psY bufs=2) and
    removes the bank-recycle stalls there.
  - All matmul operands bf16 (was float32r): no fp32r narrow-stream 4x
    penalty, FWL on stationary loads, 2x DVE modes on the exp*WT multiply.
  - Both heads packed into ONE [128,1024] AT tile per s'-tile: the exp*WT
    multiply and Z tree-add run as single wide DVE ops, halving attention
    instruction count and cross-engine handoffs (hardware charges
    ~150-200ns of sync/dispatch per instruction that the cost model does
    not show).
  - Stage-separated PSUM pools (proj/st/attn-out/z/y): a shared pool ring
    serializes pipeline stages in program order and blocks cross-chunk
    overlap.
  - Batched DMA: hsT in [128, 4x512] k-groups, WT head-interleaved in
    [128, 4x1024] t-groups, y one row-block [128, 2048] per tile; the
    attention aop is staged to SBUF immediately so its PSUM bank recycles
    without waiting for the Z chain.
  - hs/WT pre-tiled on the host into exact SBUF-tile order (hs4/wt4):
    every steady-state input DMA is one fully-contiguous DRAM block with
    4-8 KB per-partition lines (was 1 KB pieces with strided descriptors).
  - Stage-staggered emission: emission order IS per-engine execution
    order (in-order queues), and outproj(j) waits ~us on the Z/normalize
    chain of its chunk. Emitting outproj(j) AFTER proj(j+1) puts 27us of
    independent matmuls ahead of that wait, so the PE queue head always
    has ready work (hardware otherwise also pays a HAM re-throttle on
    every such stall). Finer-grained interleaving regresses: hardware
    punishes PSUM-bank switching between matmuls, so stage runs must
    stay long and bank-coherent.
"""

import math
import sys

sys.path.insert(0, "/opt/trn_rl_repo")

import numpy as np

B, S, H, D, KS = 1, 2048, 16, 128, 64
HID = H * D
NCORES = 8
HPC = H // NCORES          # heads per core
CPC = HPC * D              # output channels per core
P = 128                    # partitions
SC = 512                   # s-chunk (PSUM bank width in f32)
NJ = S // SC               # 4 s-chunks
NT = S // P                # 16 s'-tiles
NK = HID // P              # 16 contraction chunks

# dtype knobs (numpy dtype name per tensor class); PSUM is always f32.
DT_PROJ = "bfloat16"       # hsT + Wq/Wk/Wv operands of the QKV projections
DT_QK = "bfloat16"         # Q^T/K^T operands of the score matmul
DT_ATT = "bfloat16"        # exp(S)*W and V operands of the AV matmul
DT_WT = "bfloat16"         # the scattered exp(geo) tensor (DMA-heavy)
DT_WO = "bfloat16"         # AOT and Wo operands of the output projection

_CACHE = {}


def _np_dt(name):
    if name == "bfloat16":
        import ml_dtypes

        return np.dtype(ml_dtypes.bfloat16)
    if name == "float32r":
        return np.dtype(np.float32)
    return np.dtype(name)


def _my_dt(name):
    from concourse import mybir

    return {
        "float32": mybir.dt.float32,
        "float32r": mybir.dt.float32r,
        "bfloat16": mybir.dt.bfloat16,
    }[name]


def _build_nc(reps=1, loop=None, mode="full"):
    """Build the kernel NEFF. reps: python-unrolled body repetitions
    (pipelined back-to-back, used for marginal timing). loop: optional
    hardware For_i loop count around the whole body (huge amplification
    for noise-immune wall-clock timing; each iteration re-runs the
    reps-unrolled body, with an all-engine barrier between iterations)."""
    import contextlib

    import concourse.tile as tile
    from concourse import bacc, bass_isa, mybir

    F32 = mybir.dt.float32
    EXP = mybir.ActivationFunctionType.Exp
    MULT = mybir.AluOpType.mult
    ADD = mybir.AluOpType.add
    RADD = bass_isa.ReduceOp.add

    nc = bacc.Bacc("TRN2", target_bir_lowering=False, debug=False,
                   num_devices=NCORES)

    # hs4/wt4 are host-pre-tiled into exact SBUF-tile order so every
    # steady-state DMA is a fully contiguous block (4-8 KB per partition
    # line instead of 1 KB pieces): hs4[j, g] is the [P, 4*SC] hsT
    # k-group tile for chunk j; wt4[j, g] is the [P, 4t x (h, SC)]
    # head-interleaved WT t-group tile for chunk j.
    hs4 = nc.dram_tensor("hs4", [NJ, NK // 4, P, 4 * SC], _my_dt(DT_PROJ),
                         kind="ExternalInput")
    wqT = nc.dram_tensor("wqT", [HID, CPC], _my_dt(DT_PROJ), kind="ExternalInput")
    wkT = nc.dram_tensor("wkT", [HID, CPC], _my_dt(DT_PROJ), kind="ExternalInput")
    wvT = nc.dram_tensor("wvT", [HID, CPC], _my_dt(DT_PROJ), kind="ExternalInput")
    woT = nc.dram_tensor("woT", [CPC, HID], _my_dt(DT_WO), kind="ExternalInput")
    wt4 = nc.dram_tensor("wt4", [NJ, NT // 4, P, 4 * 2 * SC], _my_dt(DT_WT),
                         kind="ExternalInput")
    y = nc.dram_tensor("y", [S, HID], mybir.dt.bfloat16, kind="ExternalOutput")

    inv_sqrt_d = 1.0 / math.sqrt(D)

    def mm(out, lhsT, rhs, **kw):
        nc.tensor.matmul(out, lhsT, rhs, **kw)

    with tile.TileContext(nc) as tc, \
            nc.allow_low_precision(reason="bf16 matmul operands; PSUM accum stays f32"):
        with tc.tile_pool(name="persist", bufs=1) as persist:
            QT = [persist.tile([P, S], _my_dt(DT_QK), tag=f"qt{h}", name=f"qt{h}")
                  for h in range(HPC)]
            KT = [persist.tile([P, S], _my_dt(DT_QK), tag=f"kt{h}", name=f"kt{h}")
                  for h in range(HPC)]
            Vsb = [persist.tile([P, CPC], _my_dt(DT_ATT), tag=f"v{t}", name=f"vres{t}")
                   for t in range(NT)]
            AOT = [persist.tile([P, S], _my_dt(DT_WO), tag=f"aot{h}", name=f"aot{h}")
                   for h in range(HPC)]
            if mode == "noattn":
                for h in range(HPC):
                    nc.gpsimd.memset(AOT[h][:], 0.5)
            if mode == "noproj":
                for h in range(HPC):
                    nc.gpsimd.memset(QT[h][:], 0.01)
                    nc.gpsimd.memset(KT[h][:], 0.01)
                for t in range(NT):
                    nc.gpsimd.memset(Vsb[t][:], 0.01)
            # diagnostic modes: "dmaonly" emits only the steady-state DMA
            # streams (hs4/wt4 in, y out); "noindma" replaces hs4/wt4 DMAs
            # with fixed memset tiles but keeps all compute + y-write.
            hs_fix, wt_fix, ys_fix = [], [], []
            if mode == "noindma":
                for g in range(NK // 4):
                    t_ = persist.tile([P, 4 * SC], _my_dt(DT_PROJ),
                                      tag=f"hsfix{g}", name=f"hsfix{g}")
                    nc.gpsimd.memset(t_[:], 0.01)
                    hs_fix.append(t_)
                for g in range(NT // 4):
                    t_ = persist.tile([P, 4 * 2 * SC], _my_dt(DT_WT),
                                      tag=f"wtfix{g}", name=f"wtfix{g}")
                    nc.gpsimd.memset(t_[:], 0.01)
                    wt_fix.append(t_)
            if mode == "dmaonly":
                t_ = persist.tile([P, S], mybir.dt.bfloat16, tag="ysfix",
                                  name="ysfix")
                nc.gpsimd.memset(t_[:], 0.01)
                ys_fix.append(t_)

            # Pipelined over j-chunks of the query/sequence dim: for each j,
            # project chunk j, run attention for chunk j (both heads), then
            # the output projection for s-tiles 4j..4j+3. Chunk-level deps
            # let the Tile scheduler overlap all three stages across j.
            with tc.tile_pool(name="wpool", bufs=1) as wpool, \
                 tc.tile_pool(name="hpool", bufs=6) as hpool, \
                 tc.tile_pool(name="wop", bufs=1) as wop, \
                 tc.tile_pool(name="wtp", bufs=5) as wtp, \
                 tc.tile_pool(name="atp", bufs=10) as atp, \
                 tc.tile_pool(name="rbp", bufs=4) as rbp, \
                 tc.tile_pool(name="zacc", bufs=3) as zaccp, \
                 tc.tile_pool(name="ypool", bufs=3) as ypool, \
                 tc.tile_pool(name="psP", bufs=2, space="PSUM") as psP, \
                 tc.tile_pool(name="psS", bufs=2, space="PSUM") as psS, \
                 tc.tile_pool(name="psA", bufs=2, space="PSUM") as psA, \
                 tc.tile_pool(name="psY", bufs=2, space="PSUM") as psY:
                wq_sb, wk_sb, wv_sb = [], [], []
                wo_sb = []

                def emit_wo():
                    for h in range(HPC):
                        t_ = wop.tile([P, HID], _my_dt(DT_WO),
                                      tag=f"wo{h}", name=f"wo{h}")
                        nc.sync.dma_start(t_[:], woT[h * P:(h + 1) * P, :])
                        wo_sb.append(t_)

                def proj_start(j, first):
                    # -- QKV projection DMAs for chunk j; returns hs_t --
                    # DMA order matters for the cold start: the first
                    # Q-projection chain needs all wq chunks + all hst
                    # chunks of j=0, so those stream first; wk/wv/wo
                    # follow (they are consumed by later chains).
                    hs_t = []
                    for k in range(NK):
                        if first and mode != "dmaonly":
                            t_ = wpool.tile([P, CPC], _my_dt(DT_PROJ),
                                            tag=f"wq{k}", name=f"wq{k}")
                            nc.sync.dma_start(t_[:], wqT[k * P:(k + 1) * P, :])
                            wq_sb.append(t_)
                        if k % 4 == 0:
                            if mode == "noindma":
                                tb = hs_fix[k // 4]
                            else:
                                tb = hpool.tile([P, 4 * SC], _my_dt(DT_PROJ),
                                                tag="hst", name="hst")
                                nc.sync.dma_start(tb[:], hs4[j, k // 4])
                        hs_t.append(tb[:, (k % 4) * SC:(k % 4 + 1) * SC])
                    if mode == "dmaonly":
                        return None
                    if first:
                        for lst, dram, nm in ((wk_sb, wkT, "wk"),
                                              (wv_sb, wvT, "wv")):
                            for k in range(NK):
                                t_ = wpool.tile([P, CPC], _my_dt(DT_PROJ),
                                                tag=f"{nm}{k}", name=f"{nm}{k}")
                                nc.sync.dma_start(
                                    t_[:], dram[k * P:(k + 1) * P, :])
                                lst.append(t_)
                        emit_wo()
                    return hs_t

                def proj_qk(j, hs_t):
                    if hs_t is None:
                        return
                    for h in range(HPC):
                        for wi, (w_sb, acc) in enumerate(((wq_sb, QT),
                                                          (wk_sb, KT))):
                            pp = psP.tile([P, SC], F32, tag="big", name="ps_proj")
                            for k in range(NK):
                                mm(pp[:], w_sb[k][:, h * D:(h + 1) * D],
                                   hs_t[k][:],
                                   start=(k == 0), stop=(k == NK - 1))
                            if (2 * h + wi) % 2 == 0:
                                nc.scalar.copy(
                                    acc[h][:, j * SC:(j + 1) * SC], pp[:])
                            else:
                                nc.vector.tensor_copy(
                                    acc[h][:, j * SC:(j + 1) * SC], pp[:])

                def proj_v(j, hs_t):
                    if hs_t is None:
                        return
                    for si in range(SC // P):
                        vp = psP.tile([P, CPC], F32, tag="big", name="ps_projv")
                        for k in range(NK):
                            mm(vp[:], hs_t[k][:, si * P:(si + 1) * P],
                               wv_sb[k][:],
                               start=(k == 0), stop=(k == NK - 1))
                        nc.vector.tensor_copy(Vsb[4 * j + si][:], vp[:])

                def emit_proj(j, first):
                    hs_t = proj_start(j, first)
                    proj_qk(j, hs_t)
                    proj_v(j, hs_t)

                def attn_start(j):
                    # -- attention for chunk j. Both heads are packed into
                    # ONE wide [128, 1024] AT tile per s'-tile t (h0 cols
                    # 0:512, h1 cols 512:1024, chunk-aligned): the exp*WT
                    # multiply and the Z tree-add run as single wide DVE ops,
                    # halving instruction count and cross-engine handoffs,
                    # which is what the hardware actually charges for. WT is
                    # DMA'd head-interleaved to match. Z: DVE tree-accumulates
                    # AT into zacc; a [1,512] ones-matmul per head forms the
                    # column sum; reciprocal; a broadcast ones-matmul spreads
                    # 1/Z for the normalize multiply. --
                    tmax = min(4 * j + 3, NT - 1)
                    st = {"j": j, "tmax": tmax, "pend": [], "next_t": 0}
                    if mode != "dmaonly":
                        st["aop"] = [psA.tile([P, SC], F32, tag="ao",
                                              name=f"ao{h}")
                                     for h in range(HPC)]
                        st["zacc"] = zaccp.tile([P, 2 * SC], _my_dt(DT_ATT),
                                                tag="za", name="za")
                    wtg = {}
                    for g in range(0, tmax + 1, 4):
                        if mode == "noindma":
                            wtg[g] = wt_fix[g // 4]
                            continue
                        t_ = wtp.tile([P, 4 * 2 * SC], _my_dt(DT_WT),
                                      tag="wt", name="wt")
                        nc.sync.dma_start(t_[:], wt4[j, g // 4])
                        wtg[g] = t_
                    st["wtg"] = wtg
                    if mode == "dmaonly":
                        return None
                    return st

                def attn_drain_one(st):
                    t_, at_, o_ = st["pend"].pop(0)
                    for h in range(HPC):
                        mm(st["aop"][h][:, o_:SC],
                           Vsb[t_][:, h * D:(h + 1) * D],
                           at_[:, h * SC + o_:(h + 1) * SC],
                           start=(t_ == 0), stop=(t_ == st["tmax"]))

                def attn_tiles(st, t_hi):
                    # emit score tiles next_t..t_hi (inclusive)
                    if st is None:
                        return
                    j, tmax, pend = st["j"], st["tmax"], st["pend"]
                    for t in range(st["next_t"], t_hi + 1):
                        # within the diagonal block only columns s >= 128t
                        # are causally reachable; compute [o:SC) per head and
                        # zero-fill the acausal strip so the wide tree ops
                        # stay exact (WT is zero there anyway).
                        o = max(0, t * P - j * SC)
                        at = atp.tile([P, 2 * SC], _my_dt(DT_ATT), tag="at",
                                      name="at")
                        for h in range(HPC):
                            stp = psS.tile([P, SC], F32, tag="st", name="st")
                            mm(stp[:, o:SC], KT[h][:, t * P:(t + 1) * P],
                               QT[h][:, j * SC + o:(j + 1) * SC],
                               start=True, stop=True)
                            if o:
                                nc.gpsimd.memset(at[:, h * SC:h * SC + o], 0.0)
                            nc.scalar.activation(at[:, h * SC + o:(h + 1) * SC],
                                                 stp[:, o:SC], EXP,
                                                 scale=inv_sqrt_d)
                        g = (t // 4) * 4
                        base = (t - g) * 2 * SC
                        nc.vector.tensor_mul(at[:], at[:],
                                             st["wtg"][g][:, base:base + 2 * SC])
                        if t == 0:
                            nc.vector.tensor_copy(st["zacc"][:], at[:])
                        else:
                            nc.vector.tensor_tensor(st["zacc"][:],
                                                    st["zacc"][:], at[:], ADD)
                        pend.append((t, at, o))
                        if len(pend) >= 3:
                            attn_drain_one(st)
                    st["next_t"] = t_hi + 1

                def attn_finish(st):
                    if st is None:
                        return
                    j = st["j"]
                    attn_tiles(st, st["tmax"])
                    while st["pend"]:
                        attn_drain_one(st)
                    # Z = colsum(AT) over partitions on the (idle) GpSimd
                    # engine; the all-reduce leaves the sum broadcast on all
                    # 128 partitions, so 1/Z feeds the normalize multiply
                    # directly — no PE ones-matmuls, no psZ PSUM bank.
                    zsum = zaccp.tile([P, 2 * SC], _my_dt(DT_ATT), tag="zs",
                                      name="zs")
                    nc.gpsimd.partition_all_reduce(zsum[:], st["zacc"][:], P,
                                                   RADD)
                    rinv = rbp.tile([P, 2 * SC], _my_dt(DT_ATT), tag="ri",
                                    name="ri")
                    nc.vector.reciprocal(rinv[:], zsum[:])
                    for h in range(HPC):
                        aosb = rbp.tile([P, SC], F32, tag="aosb", name="aosb")
                        nc.scalar.copy(aosb[:], st["aop"][h][:])
                        nc.vector.tensor_tensor(
                            AOT[h][:, j * SC:(j + 1) * SC], aosb[:],
                            rinv[:, h * SC:(h + 1) * SC], MULT)

                def emit_outproj(j):
                    # -- output projection for s-tiles of chunk j --
                    for m in range(4 * j, 4 * j + 4):
                        if mode == "dmaonly":
                            nc.sync.dma_start(y[m * P:(m + 1) * P, :],
                                              ys_fix[0][:])
                            continue
                        ysb = ypool.tile([P, S], mybir.dt.bfloat16,
                                         tag="ysb", name="ysb")
                        for n in range(NJ):
                            yps = psY.tile([P, SC], F32, tag="y", name="ps_y")
                            for h in range(HPC):
                                mm(yps[:], AOT[h][:, m * P:(m + 1) * P],
                                   wo_sb[h][:, n * SC:(n + 1) * SC],
                                   start=(h == 0), stop=(h == HPC - 1))
                            if n % 2 == 0:
                                nc.scalar.copy(
                                    ysb[:, n * SC:(n + 1) * SC], yps[:])
                            else:
                                nc.vector.tensor_copy(
                                    ysb[:, n * SC:(n + 1) * SC], yps[:])
                        nc.sync.dma_start(y[m * P:(m + 1) * P, :], ysb[:])

                # Emission order IS per-engine execution order (in-order
                # queues). During an attention run the PE paces at the ACT
                # exp rate (psS banks free when exp drains), ~350ns idle
                # per tile plus HAM re-throttle risk. Fix: split each
                # chunk's attention into thirds and sandwich LARGE,
                # bank-coherent blocks of independent work between them --
                # next chunk's Q/K chains (13.6us), its V chains (6.8us),
                # and the previous chunk's output projection (6.8us). The
                # blocks stay unbroken (fine-grained 1:1 interleave
                # regresses: PSUM-bank switching between matmuls costs PE
                # micro-idles). outproj(j) also ends up a full slot after
                # attn(j), hiding the Z/normalize latency.
                pending_out = []

                def emit_rep(first, last):
                    if mode == "noproj" and first:
                        emit_wo()
                    for j in range(NJ):
                        if first and j == 0 and mode != "noproj":
                            emit_proj(0, True)
                        st = attn_start(j) if mode != "noattn" else None
                        nxt = j + 1 if j + 1 < NJ else (None if last else 0)
                        hs_t = (proj_start(nxt, False)
                                if nxt is not None and mode != "noproj"
                                else None)
                        if st is not None:
                            third = (st["tmax"] + 1 + 2) // 3
                            attn_tiles(st, third - 1)
                        if nxt is not None:
                            proj_qk(nxt, hs_t)
                        if st is not None:
                            attn_tiles(st, 2 * third - 1)
                        if nxt is not None:
                            proj_v(nxt, hs_t)
                        attn_finish(st)
                        while pending_out:
                            emit_outproj(pending_out.pop(0))
                        pending_out.append(j)
                    if last or loop is not None:
                        while pending_out:
                            emit_outproj(pending_out.pop(0))

                loop_ctx = (tc.For_i(0, loop) if loop is not None
                            else contextlib.nullcontext())
                with loop_ctx:
                    for _rep in range(reps):
                        emit_rep(_rep == 0, _rep == reps - 1)

    nc.compile()
    return nc


def _get_nc():
    if "nc" not in _CACHE:
        _CACHE["nc"] = _build_nc()
    return _CACHE["nc"]


def make_in_maps(hidden_states, idx, valid, geo_bias, Wq, Wk, Wv, Wo):
    """Host-side sharding/layout prep: returns the 8 per-core input maps."""
    hs = np.ascontiguousarray(np.asarray(hidden_states, np.float32)[0])
    idx = np.asarray(idx).astype(np.int64)
    valid = np.asarray(valid).astype(bool)

    dt_proj, dt_wo, dt_wt = _np_dt(DT_PROJ), _np_dt(DT_WO), _np_dt(DT_WT)

    hsT = np.ascontiguousarray(hs.T).astype(dt_proj)       # [HID, S]
    # pre-tile into SBUF order: hs4[j, g, p, kk, c] = hsT[128*(4g+kk)+p,
    # 512j+c]  ->  every steady-state DMA is one contiguous block.
    hs4 = np.ascontiguousarray(
        hsT.reshape(NK // 4, 4, P, NJ, SC).transpose(3, 0, 2, 1, 4)
        .reshape(NJ, NK // 4, P, 4 * SC))

    srange = np.arange(S)
    cmask = ((idx <= srange[:, None]) & valid).ravel()
    flat = (idx * S + srange[:, None]).ravel()[cmask]
    eg = np.exp(np.asarray(geo_bias, np.float64))          # [H, S, K]

    in_maps = []
    for c in range(NCORES):
        h0 = HPC * c
        sl = slice(h0 * D, (h0 + HPC) * D)
        wt_c = np.empty((HPC, S, S), dt_wt)
        for hh in range(HPC):
            wt_c[hh] = (np.bincount(flat,
                                    weights=eg[h0 + hh].ravel()[cmask],
                                    minlength=S * S)
                        .reshape(S, S).astype(dt_wt))
        # wt4[j, g, p, tt, h, c] = wt_c[h, 128*(4g+tt)+p, 512j+c]
        wt4 = np.ascontiguousarray(
            wt_c.reshape(HPC, NT // 4, 4, P, NJ, SC)
            .transpose(4, 1, 3, 2, 0, 5)
            .reshape(NJ, NT // 4, P, 4 * 2 * SC))
        in_maps.append({
            "hs4": hs4,
            "wqT": np.ascontiguousarray(np.asarray(Wq)[sl].T).astype(dt_proj),
            "wkT": np.ascontiguousarray(np.asarray(Wk)[sl].T).astype(dt_proj),
            "wvT": np.ascontiguousarray(np.asarray(Wv)[sl].T).astype(dt_proj),
            "woT": np.ascontiguousarray(np.asarray(Wo)[:, sl].T).astype(dt_wo),
            "wt4": wt4,
        })
    return in_maps


def kernel(hidden_states, idx, valid, geo_bias, Wq, Wk, Wv, Wo, bo):
    from concourse import bass_utils

    nc = _get_nc()
    in_maps = make_in_maps(hidden_states, idx, valid, geo_bias, Wq, Wk, Wv, Wo)
    res = bass_utils.run_bass_kernel_spmd(nc, in_maps,
                                          core_ids=list(range(NCORES)))
    out = np.zeros((S, HID), np.float32)
    for r in res.results:
        out += r["y"].astype(np.float32)
    out += np.asarray(bo, np.float32)
    return out.reshape(B, S, HID)

